# revision 39
# baseline (speedup 1.0000x reference)
"""Trainium2 Bass kernel for a 2-layer LIF spiking network (T=50, B=1024,
784 -> 1024 -> 10), data-parallel over batch across 8 NeuronCores.

Strategy:
  - Layer-1 matmuls (x[t] @ W1.T) have no recurrent dependency: computed in
    bulk on the PE in a "transposed" formulation out = W1 @ x[t].T so the
    hidden dim lands on partitions and layer 2 needs no transpose.
  - fp32 accuracy via 3-pass hi/lo split: x = xh(fp16) + xl(bf16 residual),
    W1*64 = Wh(fp16) + Wl(fp16 residual).  cur1*64 accumulates in PSUM as
    xh@Wh + xl@bf16(Wh) + xh@Wl.  The *64 scale keeps W fp16-splits out of
    the subnormal range; LIF state is simply kept at 64x scale (exact,
    power of two) and the mem2 output is scaled back by 1/64 on write-out.
  - K-tiling at 120 rows (matmuls with K>120 clock the PE down from 2.4
    to ~2.0 GHz, so K=120 maximizes MACs/cycle at full clock): 784 =
    6*120 + 64.  The 64-row tails are handled by K=32 matmuls pinned to a
    single 32-row group each (tile_position=(32j,0)); the four h-tiles of
    a batch use the four row groups, so their tail matmuls run
    concurrently and a batch's 24 tail matmuls cost ~6 matmul slots.
    (Tail matmuls spanning two row groups hang the PE - keep them within
    one 32-row group.)
  - LIF updates on the vector engine (3 ops/step + cheap 2x-mode mask
    inversion... k1-inversion on the scalar engine); each chunk's LIF is
    issued inside the NEXT chunk's instruction stream so psum
    evacuations on the scalar queue are never stuck behind the LIF
    chain.  Tiny layer-2 matmuls (K=1024, M=10) are 4-way column-packed
    on the PE via tile_position col groups.
  - Total vector-engine load is kept moderate: chip power is shared, and
    high DVE activity clocks the PE down from 2.4 to ~2.0 GHz.
"""

import os
import sys

import numpy as np
import ml_dtypes

sys.path.insert(0, "/opt/trn_rl_repo")

T, B, N_IN, N_HID, N_OUT = 50, 1024, 784, 1024, 10
SUPERS = (4, 8, 8, 8, 8, 8, 6)   # x-window sizes in timesteps
NCORES = 8
BS = B // NCORES            # batch shard per core = 128
KP = 120                    # full k-tile rows (>120 triggers a PE downclock)
KF = 6                      # 784 = 6 full k-tiles of 120 + tail of 64
HT = N_HID // 128           # 8 hidden tiles
SCALE = 64.0
CHUNK = 4                   # timesteps per layer-2/psum chunk (N = 512)

LAST_RESULT = None          # BassKernelResults of the last run (for test.py)


def _build_bass(b1: float, b2: float):
    import concourse.bass as bass
    from concourse import bacc
    import concourse.mybir as mybir
    import concourse.tile as tile

    f32 = mybir.dt.float32
    f16 = mybir.dt.float16
    bf16 = mybir.dt.bfloat16
    Alu = mybir.AluOpType
    Act = mybir.ActivationFunctionType

    nc = bacc.Bacc("TRN2", target_bir_lowering=False, debug=False,
                   num_devices=NCORES)

    TB = T * BS  # 6400
    # x tensors are window-major so each super's DMA reads one contiguous
    # span per partition (few large descriptors instead of ~720 small ones)
    xh_d = nc.dram_tensor("xh", [KP, KF * TB], f16, kind="ExternalInput")
    xl_d = nc.dram_tensor("xl", [KP, KF * TB], bf16, kind="ExternalInput")
    xhtd_d = nc.dram_tensor("xhtd", [128, 2 * TB], f16, kind="ExternalInput")
    xltd_d = nc.dram_tensor("xltd", [128, 2 * TB], bf16, kind="ExternalInput")
    w1h_d = nc.dram_tensor("w1h", [HT, KP, KF * 128], f16, kind="ExternalInput")
    w1l_d = nc.dram_tensor("w1l", [HT, KP, KF * 128], f16, kind="ExternalInput")
    w1hb_d = nc.dram_tensor("w1hb", [HT, KP, KF * 128], bf16, kind="ExternalInput")
    w1tf_d = nc.dram_tensor("w1tf", [128, 2, 4, 128], f16, kind="ExternalInput")
    w1tb_d = nc.dram_tensor("w1tb", [128, 2, 2, 128], bf16, kind="ExternalInput")
    w2h_d = nc.dram_tensor("w2h", [128, HT * N_OUT], f16, kind="ExternalInput")
    w2l_d = nc.dram_tensor("w2l", [128, HT * N_OUT], f16, kind="ExternalInput")
    spk_d = nc.dram_tensor("spk2o", [N_OUT, TB], f32, kind="ExternalOutput")
    mem_d = nc.dram_tensor("mem2o", [N_OUT, TB], f32, kind="ExternalOutput")

    # supers (x-window granularity): first small so the PE starts early
    supers = []
    t0 = 0
    for n in SUPERS:
        supers.append((t0, n))
        t0 += n
    assert t0 == T

    # global chunk list: (s_index, t0, csz); the final steps run as 1-step
    # chunks so the end-of-kernel LIF tail is short
    chunks = []
    for si, (s0, nsteps) in enumerate(supers):
        c0 = 0
        while c0 < nsteps:
            csz = min(CHUNK, nsteps - c0)
            if si == len(supers) - 1 and c0 + csz >= nsteps - 1:
                csz = 1
            chunks.append((si, s0 + c0, csz))
            c0 += csz

    with tile.TileContext(nc) as tc:
        with (
            tc.tile_pool(name="const", bufs=1) as cpool,
            tc.tile_pool(name="xs", bufs=2) as xpool,
            tc.tile_pool(name="cur", bufs=3) as curpool,
            tc.tile_pool(name="spk", bufs=2) as spkpool,
            tc.tile_pool(name="state", bufs=1) as stpool,
            tc.tile_pool(name="outst", bufs=2) as opool,
            tc.tile_pool(name="ps1", bufs=6, space="PSUM") as ps1pool,
            tc.tile_pool(name="ps2", bufs=2, space="PSUM") as ps2pool,
        ):
            # ---- weights (small/tail tensors first so tail matmuls can
            # start almost immediately) ----
            w1tf = cpool.tile([128, 2, 4, 128], f16)
            w1tb = cpool.tile([128, 2, 2, 128], bf16)
            w2h = cpool.tile([128, HT * N_OUT], f16)
            w2l = cpool.tile([128, HT * N_OUT], f16)
            w1h = cpool.tile([KP, HT, KF, 128], f16)
            w1l = cpool.tile([KP, HT, KF, 128], f16)
            w1hb = cpool.tile([KP, HT, KF, 128], bf16)
            nc.gpsimd.dma_start(w1tf[:], w1tf_d[:])
            nc.gpsimd.dma_start(w1tb[:], w1tb_d[:])

            # ---- x windows (per super), double buffered ----
            xwin = {}

            def issue_xwin(si):
                s0, nsteps = supers[si]
                NW = nsteps * BS
                winm = slice(KF * s0 * BS, KF * (s0 * BS + NW))
                wint = slice(2 * s0 * BS, 2 * (s0 * BS + NW))
                xhtd = xpool.tile([128, 2, NW], f16, tag="xhtd")
                xltd = xpool.tile([128, 2, NW], bf16, tag="xltd")
                xh = xpool.tile([KP, KF, NW], f16, tag="xh")
                xl = xpool.tile([KP, KF, NW], bf16, tag="xl")
                nc.gpsimd.dma_start(xhtd[:], xhtd_d[:, wint])
                nc.gpsimd.dma_start(xltd[:], xltd_d[:, wint])
                nc.sync.dma_start(xh[:], xh_d[:, winm])
                nc.sync.dma_start(xl[:], xl_d[:, winm])
                xwin[si] = (xh, xl, xhtd, xltd)

            issue_xwin(0)
            # weight mains split across gpsimd/scalar queues so startup
            # DMAs drain through several DMA engines in parallel; window 1
            # only after the first h-tiles' weights are in flight
            for h in range(HT):
                q = nc.scalar if h < 4 else nc.gpsimd
                q.dma_start(w1h[:, h], w1h_d[h])
                q.dma_start(w1hb[:, h], w1hb_d[h])
                q.dma_start(w1l[:, h], w1l_d[h])
            nc.scalar.dma_start(w2h[:], w2h_d[:])
            nc.scalar.dma_start(w2l[:], w2l_d[:])
            issue_xwin(1)

            # ---- persistent LIF state (layer-1 at 64x scale) ----
            m1 = stpool.tile([128, HT, 128], f32)   # mem1*64, free=(h, b)
            u1 = stpool.tile([128, HT, 128], f32)
            k1 = [stpool.tile([128, HT, 128], f16, name=f"k1_{i}")
                  for i in range(2)]                # keep mask ping-pong
            u2 = stpool.tile([N_OUT, 128], f32)
            k2 = [stpool.tile([N_OUT, 128], f16, name=f"k2_{i}")
                  for i in range(2)]
            mem0 = stpool.tile([N_OUT, 128], f32)   # initial mem2 (zeros)
            nc.vector.memset(m1[:], 0.0)
            nc.vector.memset(k1[0][:], 1.0)
            nc.vector.memset(mem0[:], 0.0)
            nc.vector.memset(k2[0][:], 1.0)

            b2s = float(np.float32(b2) * 64.0)      # exact: *2^6

            # per-chunk products kept for cross-chunk references
            cur = {}
            spk1 = {}
            memst = {}
            nglob = 0                               # global step counter

            def layer2_mms(ci):
                _, t0c, csz = chunks[ci]
                NC_ = csz * BS
                p2 = ps2pool.tile([128, NC_], f32, tag="p2",
                                  name=f"p2_{ci}")
                sp = spk1[ci]
                for cg in range(4):
                    po = 32 * cg
                    ip = 0
                    for h in (2 * cg, 2 * cg + 1):
                        os_ = slice(h * N_OUT, (h + 1) * N_OUT)
                        for wsb in (w2h, w2l):
                            nc.tensor.matmul(
                                p2[po:po + N_OUT, :], wsb[:, os_],
                                sp[:, h, :],
                                start=(ip == 0), stop=(ip == 3),
                                tile_position=(0, po))
                            ip += 1
                return p2

            def lif2(ci, p2):
                nonlocal nglob
                _, t0c, csz = chunks[ci]
                NC_ = csz * BS
                # sum the 4 column-group slabs
                c2 = opool.tile([N_OUT, NC_], f32, tag="c2",
                                name=f"c2_{ci}")
                nc.scalar.activation(c2[:], p2[0:N_OUT, :], Act.Copy)
                for cg in (1, 2, 3):
                    po = 32 * cg
                    nc.vector.scalar_tensor_tensor(
                        c2[:], p2[po:po + N_OUT, :], 1.0, c2[:],
                        op0=Alu.bypass, op1=Alu.add)
                spk_st = opool.tile([N_OUT, NC_], f32, tag="spkst",
                                    name=f"spkst_{ci}")
                mem_st = opool.tile([N_OUT, NC_], f32, tag="memst",
                                    name=f"memst_{ci}")
                memst[ci] = mem_st
                for j in range(csz):
                    bs = slice(j * BS, (j + 1) * BS)
                    g = nglob + j
                    prev = mem0[:] if g == 0 else (
                        memst[ci - 1][:, (chunks[ci - 1][2] - 1) * BS:]
                        if j == 0 else mem_st[:, (j - 1) * BS: j * BS])
                    kc, kn = k2[g % 2], k2[(g + 1) % 2]
                    # u2 = mem_prev*(64*b2) + cur2   (all at 64x scale)
                    nc.vector.scalar_tensor_tensor(
                        u2[:], prev, b2s, c2[:, bs], op0=Alu.mult,
                        op1=Alu.add)
                    # spk = (u2 > 64) * keep ; mem' = (u2 / 64) * keep
                    nc.vector.scalar_tensor_tensor(
                        spk_st[:, bs], u2[:], SCALE, kc[:], op0=Alu.is_gt,
                        op1=Alu.mult)
                    nc.vector.scalar_tensor_tensor(
                        mem_st[:, bs], u2[:], 1.0 / SCALE, kc[:],
                        op0=Alu.mult, op1=Alu.mult)
                    if g < T - 1:
                        # keep' = 1 - spk
                        nc.vector.tensor_scalar(kn[:], spk_st[:, bs],
                                                -1.0, 1.0, op0=Alu.mult,
                                                op1=Alu.add)
                nglob += csz
                ow = slice(t0c * BS, (t0c + csz) * BS)
                nc.sync.dma_start(spk_d[:, ow], spk_st[:])
                nc.sync.dma_start(mem_d[:, ow], mem_st[:])

            def lif1(ci):
                si, t0c, csz = chunks[ci]
                NC_ = csz * BS
                sp = spkpool.tile([128, HT, NC_], f16, tag="spk1",
                                  name=f"spk1_{ci}")
                spk1[ci] = sp
                cu = cur[ci]
                for j in range(csz):
                    g = t0c + j
                    bs = slice(j * BS, (j + 1) * BS)
                    kc, kn = k1[g % 2], k1[(g + 1) % 2]
                    # u = b1*m + cur ; m' = u*keep ; spk = (u>64)*keep
                    nc.vector.scalar_tensor_tensor(
                        u1[:], m1[:], b1, cu[:, :, bs], op0=Alu.mult,
                        op1=Alu.add)
                    if g < T - 1:
                        nc.vector.tensor_tensor(m1[:], u1[:], kc[:],
                                                op=Alu.mult)
                    nc.vector.scalar_tensor_tensor(
                        sp[:, :, bs], u1[:], SCALE, kc[:], op0=Alu.is_gt,
                        op1=Alu.mult)
                    if g < T - 1:
                        # keep' = 1 - spk   (scalar engine)
                        nc.scalar.activation(kn[:], sp[:, :, bs], Act.Copy,
                                             bias=1.0, scale=-1.0)

            def tails(ci, hb):
                """Tail matmuls for h-batch hb (h = 4*hb+j, j=0..3), each
                psum served only by K=32 matmuls at its own row group
                (32j, 0).  Starts the accumulation for the 4 psums."""
                si, t0c, csz = chunks[ci]
                s0 = supers[si][0]
                NC_ = csz * BS
                xh, xl, xhtd, xltd = xwin[si]
                co = slice((t0c - s0) * BS, (t0c - s0) * BS + NC_)
                ps = []
                for j in range(4):
                    ps.append(ps1pool.tile([128, NC_], f32, tag="p1",
                                           name=f"p1_{4 * hb + j}_{ci}"))
                for q in range(4):
                    for j in range(4):
                        r = slice(32 * j, 32 * j + 32)
                        nc.tensor.matmul(
                            ps[j][:], w1tf[r, hb, q, :], xhtd[r, q % 2, co],
                            start=(q == 0), stop=False,
                            tile_position=(32 * j, 0))
                for q in range(2):
                    for j in range(4):
                        r = slice(32 * j, 32 * j + 32)
                        nc.tensor.matmul(
                            ps[j][:], w1tb[r, hb, q, :], xltd[r, q, co],
                            start=False, stop=False,
                            tile_position=(32 * j, 0))
                return ps

            def fulls(ci, h, p):
                """18 full-K matmuls + psum evacuation for h-tile h."""
                si, t0c, csz = chunks[ci]
                s0 = supers[si][0]
                NC_ = csz * BS
                xh, xl = xwin[si][0], xwin[si][1]
                co = (t0c - s0) * BS
                ip = 0
                for wsb, xsb in ((w1h, xh), (w1hb, xl), (w1l, xh)):
                    for k in range(KF):
                        nc.tensor.matmul(
                            p[:], wsb[:, h, k, :],
                            xsb[:, k, co:co + NC_],
                            start=False, stop=(ip == 3 * KF - 1))
                        ip += 1
                # final chunk: evacs split across engines so the closing
                # LIF chain starts sooner
                if ci >= len(chunks) - 2 and h >= 4:
                    nc.vector.tensor_copy(cur[ci][:, h, :], p[:])
                else:
                    nc.scalar.activation(cur[ci][:, h, :], p[:], Act.Copy)

            # ================= main pipeline =================
            for ci, (si, t0c, csz) in enumerate(chunks):
                NC_ = csz * BS
                cur[ci] = curpool.tile([128, HT, NC_], f32, tag="cur1",
                                       name=f"cur1_{ci}")
                # prefetch next x window when entering a super's first chunk
                if t0c == supers[si][0] and si + 2 < len(supers):
                    issue_xwin(si + 2)
                # LIF of chunk ci-1 and its layer 2 ride inside chunk
                # ci's instruction stream: the scalar queue then orders
                # [evac h0,h1 (ci), k-invs (ci-1), evac h2,h3, ...] so psum
                # evacuations are never stuck behind the slow LIF chain,
                # and the PE reaches layer2(ci-1) only after 4 full groups
                # of ci, by which time spk1(ci-1) is ready.
                ps = tails(ci, 0)
                fulls(ci, 0, ps[0])
                fulls(ci, 1, ps[1])
                if ci > 0:
                    lif1(ci - 1)
                fulls(ci, 2, ps[2])
                fulls(ci, 3, ps[3])
                if ci > 0:
                    p2 = layer2_mms(ci - 1)
                    lif2(ci - 1, p2)
                ps = tails(ci, 1)
                for j in range(4):
                    fulls(ci, 4 + j, ps[j])
            # final chunk: its LIF and layer 2
            ci = len(chunks) - 1
            lif1(ci)
            p2 = layer2_mms(ci)
            lif2(ci, p2)

    nc.compile()
    return nc


def _prep_inputs(x, W1, W2):
    """Host-side layout + hi/lo splits. Returns (per-core x dicts, weights)."""
    f32 = np.float32
    # x: [T, B, N_IN] -> feature-major [N_IN, T, B]
    xt = np.ascontiguousarray(np.transpose(np.asarray(x, f32), (2, 0, 1)))
    xh_full = xt.astype(np.float16)
    xl_full = (xt - xh_full.astype(f32)).astype(ml_dtypes.bfloat16)

    xcores = []
    for c in range(NCORES):
        bs = slice(c * BS, (c + 1) * BS)
        d = {}
        for src, main_k, tail_k in ((xh_full, "xh", "xht"),
                                    (xl_full, "xl", "xlt")):
            a = src[:, :, bs].reshape(N_IN, T * BS)     # [784, 6400]
            main = a[:KF * KP].reshape(KF, KP, T * BS).transpose(1, 0, 2)
            tail = a[KF * KP:]                          # [64, 6400]
            taild = np.stack([np.tile(tail[:32], (4, 1)),
                              np.tile(tail[32:], (4, 1))], axis=1)
            # window-major flattening: [p, wi, (k|q), cols] contiguous
            mw, tw = [], []
            t0 = 0
            for n in SUPERS:
                w = slice(t0 * BS, (t0 + n) * BS)
                mw.append(main[:, :, w].reshape(KP, -1))
                tw.append(taild[:, :, w].reshape(128, -1))
                t0 += n
            d[main_k] = np.ascontiguousarray(np.concatenate(mw, axis=1))
            d[tail_k + "d"] = np.ascontiguousarray(
                np.concatenate(tw, axis=1))
        xcores.append(d)

    W1s = np.asarray(W1, f32) * f32(SCALE)          # [N_HID, N_IN]
    W1T = np.ascontiguousarray(W1s.T)               # [784, 1024]
    w1h = W1T.astype(np.float16)
    w1l = (W1T - w1h.astype(f32)).astype(np.float16)
    w1hb = w1h.astype(ml_dtypes.bfloat16)

    def w1_layout(a):
        # main [720, 1024] -> [HT, KP, KF*128] (h-major for per-h DMAs)
        m = a[:KF * KP].reshape(KF, KP, HT, 128)
        return np.ascontiguousarray(
            m.transpose(2, 1, 0, 3).reshape(HT, KP, KF * 128))

    # tail weights: h = 4*hb + j lives at rows 32j..32j+31, column-set
    # (hb, q); q indexes the 4 fp16 pieces (wh/wl x two 32-feature halves)
    # and the 2 bf16 pieces
    w1tf = np.zeros((128, 2, 4, 128), np.float16)
    w1tb = np.zeros((128, 2, 2, 128), ml_dtypes.bfloat16)
    wht = w1h[KF * KP:].reshape(64, HT, 128)   # [64, h, m]
    wlt = w1l[KF * KP:].reshape(64, HT, 128)
    whbt = w1hb[KF * KP:].reshape(64, HT, 128)
    for hb in range(2):
        for j in range(4):
            h = 4 * hb + j
            r = slice(32 * j, 32 * j + 32)
            w1tf[r, hb, 0] = wht[:32, h]
            w1tf[r, hb, 1] = wht[32:, h]
            w1tf[r, hb, 2] = wlt[:32, h]
            w1tf[r, hb, 3] = wlt[32:, h]
            w1tb[r, hb, 0] = whbt[:32, h]
            w1tb[r, hb, 1] = whbt[32:, h]

    W2s = np.asarray(W2, f32) * f32(SCALE)          # [N_OUT, N_HID]
    W2T = np.ascontiguousarray(W2s.T)               # [1024, 10]
    w2h = W2T.astype(np.float16)
    w2l = (W2T - w2h.astype(f32)).astype(np.float16)

    def w2_layout(a):
        # [1024, 10] -> [128, HT*10] with free=(h, o)
        return np.ascontiguousarray(
            a.reshape(HT, 128, N_OUT).transpose(1, 0, 2).reshape(
                128, HT * N_OUT))

    weights = {
        "w1h": w1_layout(w1h), "w1l": w1_layout(w1l),
        "w1hb": w1_layout(w1hb),
        "w1tf": w1tf, "w1tb": w1tb,
        "w2h": w2_layout(w2h), "w2l": w2_layout(w2l),
    }
    return xcores, weights


def _ensure_ntff_shim():
    """run_bass_kernel_spmd(trace) imports antenv.axon_hooks, absent in some
    images; install a graceful stand-in so tracing degrades instead of
    crashing."""
    try:
        import antenv.axon_hooks  # noqa: F401
        return
    except Exception:
        pass
    import types
    hook = None
    try:
        from trn_agent_boot.trn_boot import _ntff_profile_via_ctypes
        hook = _ntff_profile_via_ctypes("/opt/axon/libaxon_pjrt.so")
    except Exception:
        hook = None
    mod = types.ModuleType("antenv.axon_hooks")
    mod._hook = hook
    mod.get_axon_ntff_profile_hook = lambda: mod._hook
    mod.set_axon_ntff_profile_hook = lambda h: setattr(mod, "_hook", h)
    sys.modules["antenv.axon_hooks"] = mod


def kernel(x, W1, W2, beta1, beta2):
    global LAST_RESULT
    from concourse.bass_utils import run_bass_kernel_spmd

    _ensure_ntff_shim()

    b1 = float(np.clip(np.float32(beta1), 0.0, 1.0))
    b2 = float(np.clip(np.float32(beta2), 0.0, 1.0))

    xcores, weights = _prep_inputs(x, W1, W2)
    nc = _build_bass(b1, b2)

    in_maps = []
    for c in range(NCORES):
        m = dict(xcores[c])
        m.update(weights)
        in_maps.append(m)

    res = run_bass_kernel_spmd(nc, in_maps, core_ids=list(range(NCORES)))
    LAST_RESULT = res

    spk_parts, mem_parts = [], []
    for c in range(NCORES):
        r = res.results[c]
        spk_parts.append(
            r["spk2o"].reshape(N_OUT, T, BS).transpose(1, 2, 0))
        mem_parts.append(
            r["mem2o"].reshape(N_OUT, T, BS).transpose(1, 2, 0))
    spk2 = np.ascontiguousarray(np.concatenate(spk_parts, axis=1))
    mem2 = np.ascontiguousarray(np.concatenate(mem_parts, axis=1))
    return spk2, mem2
